# revision 43
# baseline (speedup 1.0000x reference)
"""AdaptiveDecayMemory kernel for 8 Trainium2 NeuronCores.

Math (per batch b):
    q = x Wq^T ; k = x Wk^T ; v = x Wv^T                       [T, D]
    scores[i,j] = (q[i].k[j]) / sqrt(D)
    decay[j] = sigmoid(x[j].Wd + bd); ld[j] = log(decay[j]+1e-8)
    w[i,j] = exp(ld[j] * max(j-i-1, 0)) * (j > i)
    out = ((scores*w) v) Wo^T * out_scale

Projection folding: scores = x (Wq^T Wk) x^T and
out = (S x)(Wv^T Wo^T) * out_scale, so with M = Wq^T Wk and
N = Wv^T Wo^T (each one 1024^3 matmul, batch-independent) the K and V
projections vanish: keys/values are x itself, shipped pre-cast to bf16
in both feature-major (scores lhsT) and token-major (retrieve lhsT)
layouts.  Per-core tensor stream drops from ~240us to ~190us.

All inputs ship bf16 except nothing: weights are only used inside
fp32-accumulated matmuls, and the bf16 rounding of x/M/N adds ~1e-3
relative error against a 2e-2 budget.  N stays f32r on-chip (psum copy)
so the final projection is full precision; decay logits come from the
bf16 x copy.

Sharding: data-parallel over B (4 batches) x 2-way split of query rows.
Core c handles batch b = c//2 and query-row blocks (256 rows each)
m in {0,2,5,7} (c even) or {1,3,4,6} (c odd).  This interleaving makes the
upper-triangular (j > i) attention work identical across cores, so all 8
cores run one SPMD graph: position k always scans key blocks r in
[J[k], 16) with J = [0,4,8,12], which covers j > i for both cores' block
at that position; the remainder is masked to zero on device.

Layouts on device (per core):
    xTb[e, j]  (feature-major bf16)  -> lhsT for scores^T tiles, decay rhs
    xtok[j, e] (token-major bf16)    -> lhsT for retrieved^T accumulation
    ST[j, i] tiles [128, 256] in PSUM; decay weights applied with j on
    partitions (per-partition ld scale on the scalar engine).
"""

import numpy as np

B, T, D = 4, 2048, 1024
P = 128
KD = D // P           # 8 chunks of the contraction dim
NJ = T // P           # 16 key blocks
NCORES = 8
NPOS = 4              # 256-row query blocks per core
JSTART = [0, 4, 8, 12]  # first key block (of 128) scanned at position k
MLIST = {0: [0, 2, 5, 7], 1: [1, 3, 4, 6]}  # global 256-row block ids
NMASK = 4             # tiles per position that need the j>i mask
FAR = [6, 10, 14, 16]  # key blocks r >= FAR[k] run in fp8 (min dist >= 256)
FP8_SCL = 256.0       # scale keeping fp8 scores out of e4m3 subnormals

_cache = {}


def _build_nc():
    import concourse.mybir as mybir
    from concourse import bacc
    import concourse.tile as tile

    f32 = mybir.dt.float32
    f32r = mybir.dt.float32r
    bf16 = mybir.dt.bfloat16
    Alu = mybir.AluOpType
    ACT = mybir.ActivationFunctionType

    nc = bacc.Bacc("TRN2", target_bir_lowering=False, debug=False,
                   num_devices=NCORES)

    # Weights feature-major over the *output* feature e (contraction dim
    # of M = Wq^T Wk and N = Wv^T Wo^T), bf16, in quarter tiles.
    wq_d = nc.dram_tensor("Wqe", [4, P, KD, 256], bf16, kind="ExternalInput")
    wk_d = nc.dram_tensor("Wke", [4, P, KD, 256], bf16, kind="ExternalInput")
    wv_d = nc.dram_tensor("Wve", [4, P, KD, 256], bf16, kind="ExternalInput")
    wo_d = nc.dram_tensor("Wote", [4, P, KD, 256], bf16, kind="ExternalInput")
    f8 = mybir.dt.float8e4
    # x bf16: feature-major (keys/decay), token-major (values), and the
    # core's query rows feature-major (q~ rhs).  The fp8 copies serve the
    # far-distance attention tiles (DoubleRow double-pumped matmuls).
    xTb_d = nc.dram_tensor("xTb", [P, KD, T], bf16, kind="ExternalInput")
    xtok_d = nc.dram_tensor("xtok", [P, NJ, D], bf16, kind="ExternalInput")
    xTf8_d = nc.dram_tensor("xTf8", [P, KD, T], f8, kind="ExternalInput")
    xtf8_d = nc.dram_tensor("xtf8", [P, NJ, D], f8, kind="ExternalInput")
    xq_d = nc.dram_tensor("xq", [2, P, KD, 512], bf16, kind="ExternalInput")
    wd_d = nc.dram_tensor("Wdt", [P, KD, 2], bf16, kind="ExternalInput")
    negI_d = nc.dram_tensor("negI", [P, NPOS * 256], f32, kind="ExternalInput")
    bd_d = nc.dram_tensor("bd128", [P, 1], f32, kind="ExternalInput")
    os_d = nc.dram_tensor("os128", [P, 1], f32, kind="ExternalInput")
    out_d = nc.dram_tensor("out", [NPOS * 256 // P, 2, P, 512], f32,
                           kind="ExternalOutput")

    blo2 = nc.dram_tensor("blo2", [T], f32)

    with tile.TileContext(nc) as tc:
        with (
            tc.tile_pool(name="resident", bufs=1) as res,
            tc.tile_pool(name="small", bufs=1) as small,
            tc.tile_pool(name="ldrow", bufs=2) as ldrow_pool,
            tc.tile_pool(name="proj_ps", bufs=2, space="PSUM") as proj_ps,
            tc.tile_pool(name="st_ps", bufs=2, space="PSUM") as st_ps,
            tc.tile_pool(name="ret_ps", bufs=2, space="PSUM") as ret_ps,
            tc.tile_pool(name="out_ps", bufs=2, space="PSUM") as out_ps,
        ):
            xTb = res.tile([P, KD, T], bf16)          # 32KB/part
            qT = res.tile([P, KD, NPOS * 256], bf16)  # 16KB/part
            nT = res.tile([P, KD, D], f32r)           # 32KB/part
            negI = res.tile([P, NPOS * 256], f32)     # 4KB/part
            xTf8 = res.tile([P, KD, T], f8)           # 8KB/part
            qT8 = res.tile([P, KD, NPOS * 256], f8)   # 4KB/part

            wd_t = small.tile([P, KD, 2], bf16, tag="wd")
            nc.sync.dma_start(wd_t[:], wd_d.ap())
            bd_t = small.tile([P, 1], f32, tag="bd")
            nc.sync.dma_start(bd_t[:], bd_d.ap())
            os_t = small.tile([P, 1], f32, tag="os")
            nc.sync.dma_start(os_t[:], os_d.ap())
            jall = small.tile([P, NJ], f32, tag="jall")
            nc.gpsimd.iota(jall[:], pattern=[[P, NJ]], base=0,
                           channel_multiplier=1,
                           allow_small_or_imprecise_dtypes=True)
            ldc = small.tile([P, NJ], f32, tag="ldc")
            ldT = small.tile([P, NJ], f32, tag="ldT")
            negLd = small.tile([P, NJ], f32, tag="negld")
            eps_t = small.tile([P, 1], f32, tag="eps")
            nc.vector.memset(eps_t[:], 1e-8)
            os32_t = small.tile([P, 1], f32, tag="os32")
            nc.vector.tensor_scalar_mul(os32_t[:], os_t[:],
                                        1.0 / float(np.sqrt(D)))
            # fp8 q~ carries an extra 2^8 to stay clear of e4m3 subnormals;
            # the near path gets the same 2^8 inside the exp bias and the
            # retrieve psum copy divides it back out.
            os32s_t = small.tile([P, 1], f32, tag="os32s")
            nc.vector.tensor_scalar_mul(os32s_t[:], os_t[:],
                                        float(FP8_SCL) / float(np.sqrt(D)))

            # ---- Phase 1: M = Wq^T Wk, decay, q~ = xq M, N = Wv^T Wo^T --
            with (
                tc.tile_pool(name="wstat", bufs=4) as wstat,
                tc.tile_pool(name="wrhs", bufs=4) as wrhs,
                tc.tile_pool(name="mres", bufs=1) as mres,
                tc.tile_pool(name="stage", bufs=2) as stage,
            ):
                mT = mres.tile([P, KD, D], bf16)

                def w_quarter(dram, qi):
                    t = wstat.tile([P, KD, 256], bf16, tag="wstat")
                    nc.sync.dma_start(t[:], dram.ap()[qi])
                    return t

                def rhs_quarter(dram, qi):
                    t = wrhs.tile([P, KD, 256], bf16, tag="wrhs")
                    nc.sync.dma_start(t[:], dram.ap()[qi])
                    return t

                def w_quarter_g(dram, qi):
                    # gpsimd-queue DMA: doesn't head-block the sync queue
                    t = wstat.tile([P, KD, 256], bf16, tag="wstat")
                    nc.gpsimd.dma_start(t[:], dram.ap()[qi])
                    return t

                def rhs_quarter_g(dram, qi):
                    t = wrhs.tile([P, KD, 256], bf16, tag="wrhs")
                    nc.gpsimd.dma_start(t[:], dram.ap()[qi])
                    return t

                def rhs_quarter_s(dram, qi):
                    # scalar-queue DMA (idle early): prefetch M's rhs
                    t = wrhs.tile([P, KD, 256], bf16, tag="wrhs")
                    nc.scalar.dma_start(t[:], dram.ap()[qi])
                    return t

                # M: stationary Wq (all 4 quarters live), rhs Wk streamed.
                # The first pair arrives in interleaved halves (2KB lines)
                # so neither blocks fully behind the other.
                wq_q0 = wstat.tile([P, KD, 256], bf16, tag="wstat")
                wk_q0 = wrhs.tile([P, KD, 256], bf16, tag="wrhs")
                for hh in range(2):
                    hsl = slice(hh * 4, hh * 4 + 4)
                    nc.sync.dma_start(wq_q0[:, hsl, :],
                                      wq_d.ap()[0][:, hsl, :])
                    nc.sync.dma_start(wk_q0[:, hsl, :],
                                      wk_d.ap()[0][:, hsl, :])
                wq_q = [wq_q0]
                wq_q += [w_quarter(wq_d, i) for i in range(1, 4)]
                wk_rest = [rhs_quarter_s(wk_d, i) for i in range(1, 4)]
                # bulk x loads queue on sync BEHIND the weights: they can't
                # crowd M's quarters off the DMA engines, yet still land
                # long before the decay matmuls need them
                nc.sync.dma_start(xTb[:], xTb_d.ap())
                nc.sync.dma_start(xTf8[:], xTf8_d.ap())
                nc.sync.dma_start(negI[:], negI_d.ap())
                for cq in range(4):
                    rq = wk_q0 if cq == 0 else wk_rest[cq - 1]
                    for dch in range(KD):
                        ps = proj_ps.tile([P, 256], f32, tag="proj")
                        wsl = (dch % 2) * P
                        for ech in range(KD):
                            nc.tensor.matmul(
                                ps[:], wq_q[dch // 2][:, ech, wsl:wsl + P],
                                rq[:, ech, :],
                                start=(ech == 0), stop=(ech == KD - 1))
                        nc.vector.tensor_copy(
                            mT[:, dch, cq * 256:(cq + 1) * 256], ps[:])


                # stage N's weights now (gpsimd queue) so the N matmuls
                # aren't starved behind the sync queue
                wv_q = [w_quarter_g(wv_d, i) for i in range(4)]
                wo_q01 = [rhs_quarter_g(wo_d, 0), rhs_quarter_g(wo_d, 1)]

                # decay logits from the bf16 x copy (row form via DRAM,
                # read back in column form below)
                for c in range(4):
                    lp = st_ps.tile([P, 512], f32, tag="st")
                    for od in range(KD):
                        nc.tensor.matmul(lp[0:2, :], wd_t[:, od, :],
                                         xTb[:, od, c * 512:(c + 1) * 512],
                                         start=(od == 0), stop=(od == KD - 1))
                    lr = ldrow_pool.tile([1, 512], f32, tag="lr")
                    nc.vector.tensor_copy(lr[:], lp[0:1, :])
                    nc.scalar.dma_start(blo2.ap()[c * 512:(c + 1) * 512],
                                        lr[:])

                # q~ = xq M (stationary M chunks, rhs xq stages),
                # fold out_scale/sqrt(D) into q~.
                for c in range(2):
                    xs = stage.tile([P, KD, 512], bf16, tag="xs")
                    nc.sync.dma_start(xs[:], xq_d.ap()[c])
                    for fch in range(KD):
                        ps = proj_ps.tile([P, 512], f32, tag="proj")
                        for dch in range(KD):
                            nc.tensor.matmul(
                                ps[:], mT[:, dch, fch * P:(fch + 1) * P],
                                xs[:, dch, :],
                                start=(dch == 0), stop=(dch == KD - 1))
                        nc.scalar.activation(qT[:, fch, c * 512:(c + 1) * 512],
                                             ps[:], ACT.Copy, bias=0.0,
                                             scale=os32_t[:])
                        nc.vector.tensor_scalar(
                            qT8[:, fch, c * 512:(c + 1) * 512], ps[:],
                            os32s_t[:], None, Alu.mult)

                # N: stationary Wv (reuses the wstat ring), rhs Wo^T.
                for gq in range(4):
                    rq = wo_q01[gq] if gq < 2 else rhs_quarter_g(wo_d, gq)
                    for cch in range(KD):
                        ps = proj_ps.tile([P, 256], f32, tag="proj")
                        wsl = (cch % 2) * P
                        for ech in range(KD):
                            nc.tensor.matmul(
                                ps[:], wv_q[cch // 2][:, ech, wsl:wsl + P],
                                rq[:, ech, :],
                                start=(ech == 0), stop=(ech == KD - 1))
                        nc.vector.tensor_copy(
                            nT[:, cch, gq * 256:(gq + 1) * 256], ps[:])

            # decay logits readback (column form) + decay math
            nc.scalar.dma_start(ldc[:],
                                blo2.ap().rearrange("(o p) -> p o", p=P))
            nc.scalar.activation(ldT[:], ldc[:], ACT.Sigmoid,
                                 bias=bd_t[:], scale=1.0)
            nc.scalar.activation(ldT[:], ldT[:], ACT.Ln, bias=eps_t[:])
            nc.vector.tensor_scalar_mul(negLd[:], ldT[:], -1.0)
            negLdS = small.tile([P, NJ], f32, tag="negldS")
            nc.vector.tensor_scalar(negLdS[:], negLd[:],
                                    float(np.log(FP8_SCL)), None, Alu.add)
            # ldj[p,r] = ld*(j-1) with j = 128r+p: bias for the one-pass
            # weight activation exp(ld*(j-1) + ld*(-i)) on non-mask tiles
            ldj = small.tile([P, NJ], f32, tag="ldj")
            nc.vector.tensor_scalar(ldj[:], jall[:], -1.0, None, Alu.add)
            nc.vector.tensor_mul(ldj[:], ldj[:], ldT[:])
            ldjS = small.tile([P, NJ], f32, tag="ldjS")
            nc.vector.tensor_scalar(ldjS[:], ldj[:],
                                    float(np.log(FP8_SCL)), None, Alu.add)

            # ---- Phase 2: attention + output projection per position ----
            with (
                tc.tile_pool(name="xres", bufs=1) as xres,
                tc.tile_pool(name="spool", bufs=12) as spool,
                tc.tile_pool(name="s8pool", bufs=8) as s8pool,
                tc.tile_pool(name="dwpool", bufs=4) as dwpool,
                tc.tile_pool(name="mpool", bufs=1) as mpool,
                tc.tile_pool(name="rtpool", bufs=2) as rtpool,
                tc.tile_pool(name="opool", bufs=2) as opool,
            ):
                xtok = xres.tile([P, NJ, D], bf16)    # 32KB/part
                xtf8 = xres.tile([P, NJ, D], f8)      # 8KB/part
                nc.sync.dma_start(xtok[:], xtok_d.ap())
                nc.sync.dma_start(xtf8[:], xtf8_d.ap())
                DR = mybir.MatmulPerfMode.DoubleRow

                for k in range(NPOS):
                    isl = slice(k * 256, (k + 1) * 256)
                    near_rs = list(range(JSTART[k], FAR[k]))
                    far_rs = list(range(FAR[k], NJ))
                    s_near = {}
                    s_far = {}   # pair index u -> [P, 2, 256] fp8 tile
                    for t_idx, r in enumerate(near_rs + far_rs):
                        far = r >= FAR[k]
                        # alternate psum pools: proj_ps is idle in phase 2,
                        # so scores get 4 banks of pipeline depth
                        if t_idx % 2 == 0:
                            ps = st_ps.tile([P, 256], f32, tag="st")
                        else:
                            ps = proj_ps.tile([P, 256], f32, tag="proj")
                        if far:
                            for h4 in range(4):
                                nc.tensor.matmul(
                                    ps[:],
                                    xTf8[:, 2 * h4:2 * h4 + 2,
                                         r * P:(r + 1) * P],
                                    qT8[:, 2 * h4:2 * h4 + 2, isl],
                                    start=(h4 == 0), stop=(h4 == 3),
                                    perf_mode=DR)
                        else:
                            for oe in range(KD):
                                nc.tensor.matmul(
                                    ps[:], xTb[:, oe, r * P:(r + 1) * P],
                                    qT[:, oe, isl],
                                    start=(oe == 0), stop=(oe == KD - 1))
                        # decay weights w = exp(ld*(j-i-1)), mask j > i.
                        # Mask tiles need the max/is_ge pair; non-mask tiles
                        # have j > i everywhere so the whole weight is one
                        # activation reading negI with bias ld*(j-1).
                        # near tiles fold the fp8 2^8 into the exp bias so
                        # both paths produce s at 2^8 * true scale.
                        dw = dwpool.tile([P, 256], f32, tag="dw")
                        if t_idx < NMASK:
                            nc.vector.tensor_scalar(dw[:], negI[:, isl],
                                                    jall[:, r:r + 1], 0.0,
                                                    Alu.add, Alu.max)
                            mk = mpool.tile([P, 256], f32, tag="mk")
                            nc.vector.tensor_scalar(mk[:], dw[:], 1.0, None,
                                                    Alu.is_ge)
                            nc.scalar.activation(dw[:], dw[:], ACT.Exp,
                                                 bias=negLdS[:, r:r + 1],
                                                 scale=ldT[:, r:r + 1])
                            nc.vector.tensor_mul(dw[:], dw[:], mk[:])
                        else:
                            bias_t = ldj if far else ldjS
                            nc.scalar.activation(dw[:], negI[:, isl],
                                                 ACT.Exp,
                                                 bias=bias_t[:, r:r + 1],
                                                 scale=ldT[:, r:r + 1])
                        if far:
                            u, slot = divmod(r - FAR[k], 2)
                            if slot == 0:
                                s_far[u] = s8pool.tile([P, 2, 256], f8,
                                                       tag="s8",
                                                       name=f"s8_{k}_{u}")
                            nc.vector.tensor_mul(s_far[u][:, slot, :],
                                                 ps[:], dw[:])
                        else:
                            s_sb = spool.tile([P, 256], bf16, tag="s")
                            nc.vector.tensor_mul(s_sb[:], ps[:], dw[:])
                            s_near[r] = s_sb

                    npair = len(far_rs) // 2
                    nstep = len(near_rs) + npair
                    rt = rtpool.tile([P, KD, 256], f32r, tag="rt")
                    for od in range(KD):
                        rp = ret_ps.tile([P, 256], f32, tag="ret")
                        step = 0
                        for r in near_rs:
                            nc.tensor.matmul(
                                rp[:], xtok[:, r, od * P:(od + 1) * P],
                                s_near[r][:],
                                start=(step == 0), stop=(step == nstep - 1))
                            step += 1
                        for u in range(npair):
                            r0 = FAR[k] + 2 * u
                            nc.tensor.matmul(
                                rp[:],
                                xtf8[:, r0:r0 + 2, od * P:(od + 1) * P],
                                s_far[u][:],
                                start=(step == 0), stop=(step == nstep - 1),
                                perf_mode=DR)
                            step += 1
                        nc.vector.tensor_scalar(rt[:, od, :], rp[:],
                                                1.0 / float(FP8_SCL), None,
                                                Alu.mult)

                    for isub in range(2):
                        for ec in range(2):
                            op = out_ps.tile([P, 512], f32, tag="op")
                            for od in range(KD):
                                nc.tensor.matmul(
                                    op[:], rt[:, od, isub * P:(isub + 1) * P],
                                    nT[:, od, ec * 512:(ec + 1) * 512],
                                    start=(od == 0), stop=(od == KD - 1))
                            ob = opool.tile([P, 512], f32, tag="ob")
                            nc.vector.tensor_copy(ob[:], op[:])
                            nc.sync.dma_start(
                                out_d.ap()[2 * k + isub, ec], ob[:])

    nc.compile()
    return nc


def _core_rows(h):
    return np.concatenate(
        [np.arange(256 * m, 256 * (m + 1)) for m in MLIST[h]])


def _dmalayout(arrT, ch=512):
    """[D, ncols] feature-major array -> [ncols//ch, 128, D//128, ch]."""
    d, ncols = arrT.shape
    return np.ascontiguousarray(
        arrT.reshape(d // P, P, ncols // ch, ch).transpose(2, 1, 0, 3))


def make_in_maps(x, Wq, Wk, Wv, Wo, Wd, bd, out_scale):
    import ml_dtypes
    f = np.float32
    bf = ml_dtypes.bfloat16
    x = np.asarray(x, f)
    # contraction of M/N runs over the torch-Linear *output* feature e,
    # i.e. the weights' first axis: ship them untransposed (Wo transposed).
    wqe = _dmalayout(np.asarray(Wq, f), ch=256).astype(bf)
    wke = _dmalayout(np.asarray(Wk, f), ch=256).astype(bf)
    wve = _dmalayout(np.asarray(Wv, f), ch=256).astype(bf)
    wote = _dmalayout(np.asarray(Wo, f).T, ch=256).astype(bf)
    wdt = np.ascontiguousarray(
        np.concatenate([np.asarray(Wd, f).reshape(1, D).T,
                        np.zeros((D, 1), f)], axis=1)
        .reshape(KD, P, 2).swapaxes(0, 1)).astype(bf)
    bd128 = np.full((P, 1), np.asarray(bd, f).reshape(-1)[0], f)
    os128 = np.full((P, 1), np.asarray(out_scale, f).reshape(-1)[0], f)

    in_maps = []
    rows_h = {h: _core_rows(h) for h in (0, 1)}
    negI_h = {h: np.tile(-rows_h[h].astype(f)[None, :], (P, 1))
              for h in (0, 1)}
    f8 = ml_dtypes.float8_e4m3
    for c in range(NCORES):
        b, h = c // 2, c % 2
        xb = x[b]
        xbT = xb.T  # [D, T]
        xTl = np.ascontiguousarray(xbT.reshape(KD, P, T).swapaxes(0, 1))
        xtl = np.ascontiguousarray(xb.reshape(T // P, P, D).swapaxes(0, 1))
        in_maps.append({
            "xTb": xTl.astype(bf),
            "xtok": xtl.astype(bf),
            "xTf8": xTl.astype(f8),
            "xtf8": xtl.astype(f8),
            "xq": _dmalayout(np.ascontiguousarray(xb[rows_h[h]].T)).astype(bf),
            "Wqe": wqe, "Wke": wke, "Wve": wve, "Wote": wote, "Wdt": wdt,
            "negI": negI_h[h], "bd128": bd128, "os128": os128,
        })
    return in_maps, rows_h


def assemble_out(results, rows_h):
    f = np.float32
    out = np.empty((B, T, D), f)
    for c in range(NCORES):
        b, h = c // 2, c % 2
        oc = results[c]["out"]  # [8, 2, 128, 512]
        out[b][rows_h[h]] = oc.transpose(0, 2, 1, 3).reshape(NPOS * 256, D)
    return out


def kernel(x, Wq, Wk, Wv, Wo, Wd, bd, out_scale):
    from concourse.bass_utils import run_bass_kernel_spmd

    if "nc" not in _cache:
        _cache["nc"] = _build_nc()
    nc = _cache["nc"]

    in_maps, rows_h = make_in_maps(x, Wq, Wk, Wv, Wo, Wd, bd, out_scale)
    res = run_bass_kernel_spmd(nc, in_maps, list(range(NCORES)))
    return assemble_out(res.results, rows_h)


# revision 44
# speedup vs baseline: 1.0148x; 1.0148x over previous
"""AdaptiveDecayMemory kernel for 8 Trainium2 NeuronCores.

Math (per batch b):
    q = x Wq^T ; k = x Wk^T ; v = x Wv^T                       [T, D]
    scores[i,j] = (q[i].k[j]) / sqrt(D)
    decay[j] = sigmoid(x[j].Wd + bd); ld[j] = log(decay[j]+1e-8)
    w[i,j] = exp(ld[j] * max(j-i-1, 0)) * (j > i)
    out = ((scores*w) v) Wo^T * out_scale

Projection folding: scores = x (Wq^T Wk) x^T and
out = (S x)(Wv^T Wo^T) * out_scale, so with M = Wq^T Wk and
N = Wv^T Wo^T (each one 1024^3 matmul, batch-independent) the K and V
projections vanish: keys/values are x itself, shipped pre-cast to bf16
in both feature-major (scores lhsT) and token-major (retrieve lhsT)
layouts.  Per-core tensor stream drops from ~240us to ~190us.

All inputs ship bf16 except nothing: weights are only used inside
fp32-accumulated matmuls, and the bf16 rounding of x/M/N adds ~1e-3
relative error against a 2e-2 budget.  N stays f32r on-chip (psum copy)
so the final projection is full precision; decay logits come from the
bf16 x copy.

Sharding: data-parallel over B (4 batches) x 2-way split of query rows.
Core c handles batch b = c//2 and query-row blocks (256 rows each)
m in {0,2,5,7} (c even) or {1,3,4,6} (c odd).  This interleaving makes the
upper-triangular (j > i) attention work identical across cores, so all 8
cores run one SPMD graph: position k always scans key blocks r in
[J[k], 16) with J = [0,4,8,12], which covers j > i for both cores' block
at that position; the remainder is masked to zero on device.

Layouts on device (per core):
    xTb[e, j]  (feature-major bf16)  -> lhsT for scores^T tiles, decay rhs
    xtok[j, e] (token-major bf16)    -> lhsT for retrieved^T accumulation
    ST[j, i] tiles [128, 256] in PSUM; decay weights applied with j on
    partitions (per-partition ld scale on the scalar engine).
"""

import numpy as np

B, T, D = 4, 2048, 1024
P = 128
KD = D // P           # 8 chunks of the contraction dim
NJ = T // P           # 16 key blocks
NCORES = 8
NPOS = 4              # 256-row query blocks per core
JSTART = [0, 4, 8, 12]  # first key block (of 128) scanned at position k
MLIST = {0: [0, 2, 5, 7], 1: [1, 3, 4, 6]}  # global 256-row block ids
NMASK = 4             # tiles per position that need the j>i mask
FAR = [6, 10, 14, 16]  # key blocks r >= FAR[k] run in fp8 (min dist >= 256)
FP8_SCL = 256.0       # scale keeping fp8 scores out of e4m3 subnormals

_cache = {}


def _build_nc():
    import concourse.mybir as mybir
    from concourse import bacc
    import concourse.tile as tile

    f32 = mybir.dt.float32
    f32r = mybir.dt.float32r
    bf16 = mybir.dt.bfloat16
    Alu = mybir.AluOpType
    ACT = mybir.ActivationFunctionType

    nc = bacc.Bacc("TRN2", target_bir_lowering=False, debug=False,
                   num_devices=NCORES)

    # Weights feature-major over the *output* feature e (contraction dim
    # of M = Wq^T Wk and N = Wv^T Wo^T), bf16, in quarter tiles.
    wq_d = nc.dram_tensor("Wqe", [4, P, KD, 256], bf16, kind="ExternalInput")
    wk_d = nc.dram_tensor("Wke", [4, P, KD, 256], bf16, kind="ExternalInput")
    wv_d = nc.dram_tensor("Wve", [4, P, KD, 256], bf16, kind="ExternalInput")
    wo_d = nc.dram_tensor("Wote", [4, P, KD, 256], bf16, kind="ExternalInput")
    f8 = mybir.dt.float8e4
    # x bf16: feature-major (keys/decay), token-major (values), and the
    # core's query rows feature-major (q~ rhs).  The fp8 copies serve the
    # far-distance attention tiles (DoubleRow double-pumped matmuls).
    xTb_d = nc.dram_tensor("xTb", [P, KD, T], bf16, kind="ExternalInput")
    xtok_d = nc.dram_tensor("xtok", [P, NJ, D], bf16, kind="ExternalInput")
    xTf8_d = nc.dram_tensor("xTf8", [P, KD, T], f8, kind="ExternalInput")
    xtf8_d = nc.dram_tensor("xtf8", [P, NJ, D], f8, kind="ExternalInput")
    xq_d = nc.dram_tensor("xq", [2, P, KD, 512], bf16, kind="ExternalInput")
    wd_d = nc.dram_tensor("Wdt", [P, KD, 2], bf16, kind="ExternalInput")
    negI_d = nc.dram_tensor("negI", [P, NPOS * 256], f32, kind="ExternalInput")
    bd_d = nc.dram_tensor("bd128", [P, 1], f32, kind="ExternalInput")
    os_d = nc.dram_tensor("os128", [P, 1], f32, kind="ExternalInput")
    out_d = nc.dram_tensor("out", [NPOS * 256 // P, 2, P, 512], f32,
                           kind="ExternalOutput")

    blo2 = nc.dram_tensor("blo2", [T], f32)

    with tile.TileContext(nc) as tc:
        with (
            tc.tile_pool(name="resident", bufs=1) as res,
            tc.tile_pool(name="stage", bufs=2) as stage,
            tc.tile_pool(name="spool", bufs=12) as spool,
            tc.tile_pool(name="s8pool", bufs=8) as s8pool,
            tc.tile_pool(name="dwpool", bufs=3) as dwpool,
            tc.tile_pool(name="mpool", bufs=1) as mpool,
            tc.tile_pool(name="rtpool", bufs=1) as rtpool,
            tc.tile_pool(name="opool", bufs=2) as opool,
            tc.tile_pool(name="small", bufs=1) as small,
            tc.tile_pool(name="ldrow", bufs=2) as ldrow_pool,
            tc.tile_pool(name="proj_ps", bufs=2, space="PSUM") as proj_ps,
            tc.tile_pool(name="st_ps", bufs=2, space="PSUM") as st_ps,
            tc.tile_pool(name="ret_ps", bufs=2, space="PSUM") as ret_ps,
            tc.tile_pool(name="out_ps", bufs=2, space="PSUM") as out_ps,
        ):
            xTb = res.tile([P, KD, T], bf16)          # 32KB/part
            qT = res.tile([P, KD, NPOS * 256], bf16)  # 16KB/part
            nT = res.tile([P, KD, D], f32r)           # 32KB/part
            negI = res.tile([P, NPOS * 256], f32)     # 4KB/part
            xTf8 = res.tile([P, KD, T], f8)           # 8KB/part
            qT8 = res.tile([P, KD, NPOS * 256], f8)   # 4KB/part

            wd_t = small.tile([P, KD, 2], bf16, tag="wd")
            nc.sync.dma_start(wd_t[:], wd_d.ap())
            bd_t = small.tile([P, 1], f32, tag="bd")
            nc.sync.dma_start(bd_t[:], bd_d.ap())
            os_t = small.tile([P, 1], f32, tag="os")
            nc.sync.dma_start(os_t[:], os_d.ap())
            jall = small.tile([P, NJ], f32, tag="jall")
            nc.gpsimd.iota(jall[:], pattern=[[P, NJ]], base=0,
                           channel_multiplier=1,
                           allow_small_or_imprecise_dtypes=True)
            ldc = small.tile([P, NJ], f32, tag="ldc")
            ldT = small.tile([P, NJ], f32, tag="ldT")
            negLd = small.tile([P, NJ], f32, tag="negld")
            eps_t = small.tile([P, 1], f32, tag="eps")
            nc.vector.memset(eps_t[:], 1e-8)
            os32_t = small.tile([P, 1], f32, tag="os32")
            nc.vector.tensor_scalar_mul(os32_t[:], os_t[:],
                                        1.0 / float(np.sqrt(D)))
            # fp8 q~ carries an extra 2^8 to stay clear of e4m3 subnormals;
            # the near path gets the same 2^8 inside the exp bias and the
            # retrieve psum copy divides it back out.
            os32s_t = small.tile([P, 1], f32, tag="os32s")
            nc.vector.tensor_scalar_mul(os32s_t[:], os_t[:],
                                        float(FP8_SCL) / float(np.sqrt(D)))

            # ---- Phase 1: M = Wq^T Wk, decay, q~ = xq M, N = Wv^T Wo^T --
            with (
                tc.tile_pool(name="wstat", bufs=4) as wstat,
                tc.tile_pool(name="wrhs", bufs=4) as wrhs,
                tc.tile_pool(name="mres", bufs=1) as mres,
            ):
                mT = mres.tile([P, KD, D], bf16)

                def w_quarter(dram, qi):
                    t = wstat.tile([P, KD, 256], bf16, tag="wstat")
                    nc.sync.dma_start(t[:], dram.ap()[qi])
                    return t

                def rhs_quarter(dram, qi):
                    t = wrhs.tile([P, KD, 256], bf16, tag="wrhs")
                    nc.sync.dma_start(t[:], dram.ap()[qi])
                    return t

                def w_quarter_g(dram, qi):
                    # gpsimd-queue DMA: doesn't head-block the sync queue
                    t = wstat.tile([P, KD, 256], bf16, tag="wstat")
                    nc.gpsimd.dma_start(t[:], dram.ap()[qi])
                    return t

                def rhs_quarter_g(dram, qi):
                    t = wrhs.tile([P, KD, 256], bf16, tag="wrhs")
                    nc.gpsimd.dma_start(t[:], dram.ap()[qi])
                    return t

                def rhs_quarter_s(dram, qi):
                    # scalar-queue DMA (idle early): prefetch M's rhs
                    t = wrhs.tile([P, KD, 256], bf16, tag="wrhs")
                    nc.scalar.dma_start(t[:], dram.ap()[qi])
                    return t

                # M: stationary Wq (all 4 quarters live), rhs Wk streamed.
                # The first pair arrives in interleaved halves (2KB lines)
                # so neither blocks fully behind the other.
                wq_q0 = wstat.tile([P, KD, 256], bf16, tag="wstat")
                wk_q0 = wrhs.tile([P, KD, 256], bf16, tag="wrhs")
                for hh in range(2):
                    hsl = slice(hh * 4, hh * 4 + 4)
                    nc.sync.dma_start(wq_q0[:, hsl, :],
                                      wq_d.ap()[0][:, hsl, :])
                    nc.sync.dma_start(wk_q0[:, hsl, :],
                                      wk_d.ap()[0][:, hsl, :])
                wq_q = [wq_q0]
                wq_q += [w_quarter(wq_d, i) for i in range(1, 4)]
                wk_rest = [rhs_quarter_s(wk_d, i) for i in range(1, 4)]
                # bulk x loads queue on sync BEHIND the weights: they can't
                # crowd M's quarters off the DMA engines, yet still land
                # long before the decay matmuls need them
                nc.sync.dma_start(xTb[:], xTb_d.ap())
                nc.sync.dma_start(xTf8[:], xTf8_d.ap())
                nc.sync.dma_start(negI[:], negI_d.ap())
                for cq in range(4):
                    rq = wk_q0 if cq == 0 else wk_rest[cq - 1]
                    for dch in range(KD):
                        ps = proj_ps.tile([P, 256], f32, tag="proj")
                        wsl = (dch % 2) * P
                        for ech in range(KD):
                            nc.tensor.matmul(
                                ps[:], wq_q[dch // 2][:, ech, wsl:wsl + P],
                                rq[:, ech, :],
                                start=(ech == 0), stop=(ech == KD - 1))
                        nc.vector.tensor_copy(
                            mT[:, dch, cq * 256:(cq + 1) * 256], ps[:])


                # stage N's weights now (gpsimd queue) so the N matmuls
                # aren't starved behind the sync queue
                wv_q = [w_quarter_g(wv_d, i) for i in range(4)]
                wo_q01 = [rhs_quarter_g(wo_d, 0), rhs_quarter_g(wo_d, 1)]

                # decay logits from the bf16 x copy (row form via DRAM,
                # read back in column form below)
                for c in range(4):
                    lp = st_ps.tile([P, 512], f32, tag="st")
                    for od in range(KD):
                        nc.tensor.matmul(lp[0:2, :], wd_t[:, od, :],
                                         xTb[:, od, c * 512:(c + 1) * 512],
                                         start=(od == 0), stop=(od == KD - 1))
                    lr = ldrow_pool.tile([1, 512], f32, tag="lr")
                    nc.vector.tensor_copy(lr[:], lp[0:1, :])
                    nc.scalar.dma_start(blo2.ap()[c * 512:(c + 1) * 512],
                                        lr[:])

                # q~ = xq M (stationary M chunks, rhs xq stages),
                # fold out_scale/sqrt(D) into q~.
                for c in range(2):
                    xs = stage.tile([P, KD, 512], bf16, tag="xs")
                    nc.sync.dma_start(xs[:], xq_d.ap()[c])
                    for fch in range(KD):
                        ps = proj_ps.tile([P, 512], f32, tag="proj")
                        for dch in range(KD):
                            nc.tensor.matmul(
                                ps[:], mT[:, dch, fch * P:(fch + 1) * P],
                                xs[:, dch, :],
                                start=(dch == 0), stop=(dch == KD - 1))
                        nc.scalar.activation(qT[:, fch, c * 512:(c + 1) * 512],
                                             ps[:], ACT.Copy, bias=0.0,
                                             scale=os32_t[:])
                        nc.vector.tensor_scalar(
                            qT8[:, fch, c * 512:(c + 1) * 512], ps[:],
                            os32s_t[:], None, Alu.mult)

                # N: stationary Wv (reuses the wstat ring), rhs Wo^T.
                for gq in range(4):
                    rq = wo_q01[gq] if gq < 2 else rhs_quarter_g(wo_d, gq)
                    for cch in range(KD):
                        ps = proj_ps.tile([P, 256], f32, tag="proj")
                        wsl = (cch % 2) * P
                        for ech in range(KD):
                            nc.tensor.matmul(
                                ps[:], wv_q[cch // 2][:, ech, wsl:wsl + P],
                                rq[:, ech, :],
                                start=(ech == 0), stop=(ech == KD - 1))
                        nc.vector.tensor_copy(
                            nT[:, cch, gq * 256:(gq + 1) * 256], ps[:])

            # decay logits readback (column form) + decay math
            nc.scalar.dma_start(ldc[:],
                                blo2.ap().rearrange("(o p) -> p o", p=P))
            nc.scalar.activation(ldT[:], ldc[:], ACT.Sigmoid,
                                 bias=bd_t[:], scale=1.0)
            nc.scalar.activation(ldT[:], ldT[:], ACT.Ln, bias=eps_t[:])
            nc.vector.tensor_scalar_mul(negLd[:], ldT[:], -1.0)
            negLdS = small.tile([P, NJ], f32, tag="negldS")
            nc.vector.tensor_scalar(negLdS[:], negLd[:],
                                    float(np.log(FP8_SCL)), None, Alu.add)
            # ldj[p,r] = ld*(j-1) with j = 128r+p: bias for the one-pass
            # weight activation exp(ld*(j-1) + ld*(-i)) on non-mask tiles
            ldj = small.tile([P, NJ], f32, tag="ldj")
            nc.vector.tensor_scalar(ldj[:], jall[:], -1.0, None, Alu.add)
            nc.vector.tensor_mul(ldj[:], ldj[:], ldT[:])
            ldjS = small.tile([P, NJ], f32, tag="ldjS")
            nc.vector.tensor_scalar(ldjS[:], ldj[:],
                                    float(np.log(FP8_SCL)), None, Alu.add)

            # ---- Phase 2: attention + output projection per position ----
            with tc.tile_pool(name="xres", bufs=1) as xres:
                xtok = xres.tile([P, NJ, D], bf16)    # 32KB/part
                xtf8 = xres.tile([P, NJ, D], f8)      # 8KB/part
                nc.sync.dma_start(xtok[:], xtok_d.ap())
                nc.sync.dma_start(xtf8[:], xtf8_d.ap())
                DR = mybir.MatmulPerfMode.DoubleRow

                for k in range(NPOS):
                    isl = slice(k * 256, (k + 1) * 256)
                    near_rs = list(range(JSTART[k], FAR[k]))
                    far_rs = list(range(FAR[k], NJ))
                    s_near = {}
                    s_far = {}   # pair index u -> [P, 2, 256] fp8 tile
                    for t_idx, r in enumerate(near_rs + far_rs):
                        far = r >= FAR[k]
                        # alternate psum pools: proj_ps is idle in phase 2,
                        # so scores get 4 banks of pipeline depth
                        if t_idx % 2 == 0:
                            ps = st_ps.tile([P, 256], f32, tag="st")
                        else:
                            ps = proj_ps.tile([P, 256], f32, tag="proj")
                        if far:
                            for h4 in range(4):
                                nc.tensor.matmul(
                                    ps[:],
                                    xTf8[:, 2 * h4:2 * h4 + 2,
                                         r * P:(r + 1) * P],
                                    qT8[:, 2 * h4:2 * h4 + 2, isl],
                                    start=(h4 == 0), stop=(h4 == 3),
                                    perf_mode=DR)
                        else:
                            for oe in range(KD):
                                nc.tensor.matmul(
                                    ps[:], xTb[:, oe, r * P:(r + 1) * P],
                                    qT[:, oe, isl],
                                    start=(oe == 0), stop=(oe == KD - 1))
                        # decay weights w = exp(ld*(j-i-1)), mask j > i.
                        # Mask tiles need the max/is_ge pair; non-mask tiles
                        # have j > i everywhere so the whole weight is one
                        # activation reading negI with bias ld*(j-1).
                        # near tiles fold the fp8 2^8 into the exp bias so
                        # both paths produce s at 2^8 * true scale.
                        dw = dwpool.tile([P, 256], f32, tag="dw")
                        if t_idx < NMASK:
                            nc.vector.tensor_scalar(dw[:], negI[:, isl],
                                                    jall[:, r:r + 1], 0.0,
                                                    Alu.add, Alu.max)
                            mk = mpool.tile([P, 256], f32, tag="mk")
                            nc.vector.tensor_scalar(mk[:], dw[:], 1.0, None,
                                                    Alu.is_ge)
                            nc.scalar.activation(dw[:], dw[:], ACT.Exp,
                                                 bias=negLdS[:, r:r + 1],
                                                 scale=ldT[:, r:r + 1])
                            nc.vector.tensor_mul(dw[:], dw[:], mk[:])
                        else:
                            bias_t = ldj if far else ldjS
                            nc.scalar.activation(dw[:], negI[:, isl],
                                                 ACT.Exp,
                                                 bias=bias_t[:, r:r + 1],
                                                 scale=ldT[:, r:r + 1])
                        if far:
                            u, slot = divmod(r - FAR[k], 2)
                            if slot == 0:
                                s_far[u] = s8pool.tile([P, 2, 256], f8,
                                                       tag="s8",
                                                       name=f"s8_{k}_{u}")
                            nc.vector.tensor_mul(s_far[u][:, slot, :],
                                                 ps[:], dw[:])
                        else:
                            s_sb = spool.tile([P, 256], bf16, tag="s")
                            nc.vector.tensor_mul(s_sb[:], ps[:], dw[:])
                            s_near[r] = s_sb

                    npair = len(far_rs) // 2
                    nstep = len(near_rs) + npair
                    rt = rtpool.tile([P, KD, 256], f32r, tag="rt")
                    for od in range(KD):
                        rp = ret_ps.tile([P, 256], f32, tag="ret")
                        step = 0
                        for r in near_rs:
                            nc.tensor.matmul(
                                rp[:], xtok[:, r, od * P:(od + 1) * P],
                                s_near[r][:],
                                start=(step == 0), stop=(step == nstep - 1))
                            step += 1
                        for u in range(npair):
                            r0 = FAR[k] + 2 * u
                            nc.tensor.matmul(
                                rp[:],
                                xtf8[:, r0:r0 + 2, od * P:(od + 1) * P],
                                s_far[u][:],
                                start=(step == 0), stop=(step == nstep - 1),
                                perf_mode=DR)
                            step += 1
                        nc.vector.tensor_scalar(rt[:, od, :], rp[:],
                                                1.0 / float(FP8_SCL), None,
                                                Alu.mult)

                    for isub in range(2):
                        for ec in range(2):
                            op = out_ps.tile([P, 512], f32, tag="op")
                            for od in range(KD):
                                nc.tensor.matmul(
                                    op[:], rt[:, od, isub * P:(isub + 1) * P],
                                    nT[:, od, ec * 512:(ec + 1) * 512],
                                    start=(od == 0), stop=(od == KD - 1))
                            ob = opool.tile([P, 512], f32, tag="ob")
                            nc.vector.tensor_copy(ob[:], op[:])
                            nc.sync.dma_start(
                                out_d.ap()[2 * k + isub, ec], ob[:])

    nc.compile()
    return nc


def _core_rows(h):
    return np.concatenate(
        [np.arange(256 * m, 256 * (m + 1)) for m in MLIST[h]])


def _dmalayout(arrT, ch=512):
    """[D, ncols] feature-major array -> [ncols//ch, 128, D//128, ch]."""
    d, ncols = arrT.shape
    return np.ascontiguousarray(
        arrT.reshape(d // P, P, ncols // ch, ch).transpose(2, 1, 0, 3))


def make_in_maps(x, Wq, Wk, Wv, Wo, Wd, bd, out_scale):
    import ml_dtypes
    f = np.float32
    bf = ml_dtypes.bfloat16
    x = np.asarray(x, f)
    # contraction of M/N runs over the torch-Linear *output* feature e,
    # i.e. the weights' first axis: ship them untransposed (Wo transposed).
    wqe = _dmalayout(np.asarray(Wq, f), ch=256).astype(bf)
    wke = _dmalayout(np.asarray(Wk, f), ch=256).astype(bf)
    wve = _dmalayout(np.asarray(Wv, f), ch=256).astype(bf)
    wote = _dmalayout(np.asarray(Wo, f).T, ch=256).astype(bf)
    wdt = np.ascontiguousarray(
        np.concatenate([np.asarray(Wd, f).reshape(1, D).T,
                        np.zeros((D, 1), f)], axis=1)
        .reshape(KD, P, 2).swapaxes(0, 1)).astype(bf)
    bd128 = np.full((P, 1), np.asarray(bd, f).reshape(-1)[0], f)
    os128 = np.full((P, 1), np.asarray(out_scale, f).reshape(-1)[0], f)

    in_maps = []
    rows_h = {h: _core_rows(h) for h in (0, 1)}
    negI_h = {h: np.tile(-rows_h[h].astype(f)[None, :], (P, 1))
              for h in (0, 1)}
    f8 = ml_dtypes.float8_e4m3
    for c in range(NCORES):
        b, h = c // 2, c % 2
        xb = x[b]
        xbT = xb.T  # [D, T]
        xTl = np.ascontiguousarray(xbT.reshape(KD, P, T).swapaxes(0, 1))
        xtl = np.ascontiguousarray(xb.reshape(T // P, P, D).swapaxes(0, 1))
        in_maps.append({
            "xTb": xTl.astype(bf),
            "xtok": xtl.astype(bf),
            "xTf8": xTl.astype(f8),
            "xtf8": xtl.astype(f8),
            "xq": _dmalayout(np.ascontiguousarray(xb[rows_h[h]].T)).astype(bf),
            "Wqe": wqe, "Wke": wke, "Wve": wve, "Wote": wote, "Wdt": wdt,
            "negI": negI_h[h], "bd128": bd128, "os128": os128,
        })
    return in_maps, rows_h


def assemble_out(results, rows_h):
    f = np.float32
    out = np.empty((B, T, D), f)
    for c in range(NCORES):
        b, h = c // 2, c % 2
        oc = results[c]["out"]  # [8, 2, 128, 512]
        out[b][rows_h[h]] = oc.transpose(0, 2, 1, 3).reshape(NPOS * 256, D)
    return out


def kernel(x, Wq, Wk, Wv, Wo, Wd, bd, out_scale):
    from concourse.bass_utils import run_bass_kernel_spmd

    if "nc" not in _cache:
        _cache["nc"] = _build_nc()
    nc = _cache["nc"]

    in_maps, rows_h = make_in_maps(x, Wq, Wk, Wv, Wo, Wd, bd, out_scale)
    res = run_bass_kernel_spmd(nc, in_maps, list(range(NCORES)))
    return assemble_out(res.results, rows_h)
